# revision 2
# baseline (speedup 1.0000x reference)
"""ACT-R activation recurrence kernel for 8 TRN2 NeuronCores — transposed layout.

Math (per batch column b, S=128 steps):
  m[0] = -inf;  decay[j] = c*exp(m[j]) + a
  m[i] = log(sum_{j<i} ((sp[i]-sp[j])*scale)^(-decay[j])),  scale = 86400*h
  out[i-1] = sigmoid((m[i] - tau)/s)

Layout: batch on partitions. Per core B=2048 = 128 partitions x 16 chunks; the
step axis j lives on the free axis: X[p, bc, j] ~ x[j, bc*128+p].  All engine
costs on TRN2 scale with FREE size only, so this halves elementwise cost vs
the j-on-partition layout and eliminates every matmul:
  d_i[:, :, j<i] = sp_i - sp_j        (broadcast-sub, Pool engine, f32)
  L_i = Ln(scale*d_i)                 (ACT, f16 out) — computed W steps ahead
  Q_i = L_i * negdecay[:, :, j<i]     (DVE f16, 2x mode)
  T_i = Exp(Q_i)                      (ACT f16)
  P_i = reduce_add_X(T_i[j<i-1])      (DVE, f32)  [bulk, lag-1]
  F_i = Exp(L_i[i-1]*negdecay[i-1])   (tiny [128,16,1] critical-path ops)
  negdecay[:, :, i] = -c*(P_i + F_i) - a   (fused scalar_tensor_tensor)
Final: S = (negdecay + a)/(-c); out = Sigmoid((Ln(S) - tau)/s).

Ln and Exp share one activation table (natural_log_exp_and_others — steered via
a get_activation_tables patch), so there is no per-step table reload.
No PSUM, no PE, no collectives.  Outputs are concatenated on host.
"""

import sys

for _p in ("/opt/trn_rl_repo",):
    if _p not in sys.path:
        sys.path.insert(0, _p)

import numpy as np
from contextlib import ExitStack

# --- act table patch: serve Ln and Exp from the combined table so the ---
# --- per-step Ln/Exp alternation does not thrash ACT table loads.     ---
import concourse.hw_specs as hw_specs
import concourse.bacc as bacc_mod
from concourse import mybir

_orig_gat = hw_specs.get_activation_tables


def _patched_gat(arch):
    tabs = _orig_gat(arch)
    out = {}
    ln_t = mybir.ActivationFunctionType.Ln
    exp_t = mybir.ActivationFunctionType.Exp
    for name, funcs in tabs.items():
        f = set(funcs)
        if name != "natural_log_exp_and_others":
            f.discard(ln_t)
            f.discard(exp_t)
        out[name] = f
    return out


bacc_mod.get_activation_tables = _patched_gat

import concourse.bass as bass
import concourse.bacc as bacc
import concourse.tile as tile
from concourse.bass_utils import run_bass_kernel_spmd

S = 128
B_FULL = 16384
N_CORES = 8
B = B_FULL // N_CORES  # 2048 per core
P = 128                # partitions (batch)
NC = B // P            # 16 chunks per partition

F32 = mybir.dt.float32
F16 = mybir.dt.float16
AF = mybir.ActivationFunctionType
ALU = mybir.AluOpType

W = 4      # Ln lookahead (steps)
NR = W + 1  # ring slots for D / L


def build_kernel(a, c, s, tau, h):
    scale = 86400.0 * float(h)
    nc = bacc.Bacc()

    spt_in = nc.declare_dram_parameter("spt", [P, NC, S], F32, isOutput=False)
    out_ext = nc.declare_dram_parameter("out", [P, NC, S], F32, isOutput=True)

    with ExitStack() as ctx:
        tc = ctx.enter_context(tile.TileContext(nc))
        pool = ctx.enter_context(tc.tile_pool(name="p", bufs=1))

        SPT = pool.tile([P, NC, S], F32)
        nc.sync.dma_start(out=SPT[:], in_=spt_in[:])

        # negdecay[:, :, j] = -(a + c*S_j); col 0 = -a (S_0 = exp(-inf) = 0)
        NEG = pool.tile([P, NC, S], F16)
        nc.vector.memset(NEG[:], -float(a))
        # NEGP[:, :, i] = -c*P_i - a (bulk partial, written for i >= 2;
        # memset covers i = 1 where P_1 = 0)
        NEGP = pool.tile([P, NC, S], F32)
        nc.vector.memset(NEGP[:], -float(a))

        D = [pool.tile([P, NC, S], F32, name=f"D{r}") for r in range(NR)]
        LR = [pool.tile([P, NC, S], F16, name=f"LR{r}") for r in range(NR)]
        QB = [pool.tile([P, NC, S], F16, name=f"QB{r}") for r in range(2)]
        TB = [pool.tile([P, NC, S], F16, name=f"TB{r}") for r in range(2)]
        QF = [pool.tile([P, NC, 1], F16, name=f"QF{r}") for r in range(2)]
        TF = [pool.tile([P, NC, 1], F32, name=f"TF{r}") for r in range(2)]
        PR = [pool.tile([P, NC, 1], F32, name=f"PR{r}") for r in range(2)]
        BIAS = pool.tile([P, 1], F32)
        nc.vector.memset(BIAS[:], -float(tau) / float(s))

        def emit_lookahead(i):
            # D_i[:, :, 0:i] = sp_i - sp_j ; L_i = Ln(scale * D_i)   (f16)
            r = i % NR
            b0, b1 = bass.broadcast_tensor_aps(
                SPT[:, :, i : i + 1], SPT[:, :, 0:i]
            )
            nc.gpsimd.tensor_tensor(
                out=D[r][:, :, 0:i], in0=b0, in1=b1, op=ALU.subtract
            )
            nc.scalar.activation(
                LR[r][:, :, 0:i], D[r][:, :, 0:i], AF.Ln, scale=scale
            )

        for i in range(1, min(W + 1, S)):
            emit_lookahead(i)

        for i in range(1, S):
            r = i % NR
            x = i % 2
            # --- fresh term (critical path): F_i = exp(L[i-1]*neg[i-1]) ---
            nc.vector.tensor_tensor(
                out=QF[x][:], in0=LR[r][:, :, i - 1 : i],
                in1=NEG[:, :, i - 1 : i], op=ALU.mult,
            )
            nc.scalar.activation(TF[x][:], QF[x][:], AF.Exp)
            # --- bulk for step i+1: P_{i+1} = sum_{j<i} T ---
            if i + 1 < S:
                r1 = (i + 1) % NR
                x1 = (i + 1) % 2
                nc.vector.tensor_tensor(
                    out=QB[x1][:, :, 0:i], in0=LR[r1][:, :, 0:i],
                    in1=NEG[:, :, 0:i], op=ALU.mult,
                )
                nc.scalar.activation(
                    TB[x1][:, :, 0:i], QB[x1][:, :, 0:i], AF.Exp
                )
            # --- close step i: NEG[i] = -c*TF + NEGP[i] ---
            nc.vector.scalar_tensor_tensor(
                out=NEG[:, :, i : i + 1], in0=TF[x][:], scalar=-float(c),
                in1=NEGP[:, :, i : i + 1], op0=ALU.mult, op1=ALU.add,
            )
            if i + 1 < S:
                nc.vector.tensor_reduce(
                    out=PR[x1][:], in_=TB[x1][:, :, 0:i],
                    axis=mybir.AxisListType.X, op=ALU.add,
                )
                nc.vector.tensor_scalar(
                    out=NEGP[:, :, i + 1 : i + 2], in0=PR[x1][:],
                    scalar1=-float(c), scalar2=-float(a),
                    op0=ALU.mult, op1=ALU.add,
                )
            if i + W < S:
                emit_lookahead(i + W)

        # --- epilogue: S = (NEG + a)*(-1/c); out = Sigmoid((Ln(S)-tau)/s) ---
        SS = pool.tile([P, NC, S], F32)
        nc.vector.tensor_scalar(
            out=SS[:, :, 1:S], in0=NEG[:, :, 1:S],
            scalar1=float(a), scalar2=-1.0 / float(c),
            op0=ALU.add, op1=ALU.mult,
        )
        M = pool.tile([P, NC, S], F32)
        nc.scalar.activation(M[:, :, 1:S], SS[:, :, 1:S], AF.Ln)
        O = pool.tile([P, NC, S], F32)
        nc.vector.memset(O[:, :, 0:1], 0.0)
        nc.scalar.activation(
            O[:, :, 1:S], M[:, :, 1:S], AF.Sigmoid,
            scale=1.0 / float(s), bias=BIAS[:],
        )
        nc.sync.dma_start(out=out_ext[:], in_=O[:])

    nc.compile()
    return nc


def make_in_maps(sp: np.ndarray) -> list:
    in_maps = []
    for ci in range(N_CORES):
        shard = sp[:, ci * B : (ci + 1) * B]  # [S, B]
        # spt[p, bc, j] = shard[j, bc*128 + p]
        spt = np.ascontiguousarray(
            shard.reshape(S, NC, P).transpose(2, 1, 0).astype(np.float32)
        )
        in_maps.append({"spt": spt})
    return in_maps


def kernel(sp: np.ndarray, w: np.ndarray) -> np.ndarray:
    sp = np.ascontiguousarray(sp, dtype=np.float32)
    w = np.asarray(w, dtype=np.float32)
    a, c, s, tau, h = (float(x) for x in w)

    nc = build_kernel(a, c, s, tau, h)
    in_maps = make_in_maps(sp)

    res = run_bass_kernel_spmd(nc, in_maps, core_ids=list(range(N_CORES)))
    outs = []
    for ci in range(N_CORES):
        o = res.results[ci]["out"]  # [P, NC, S]
        outs.append(o.transpose(2, 1, 0).reshape(S, B)[1:S])
    return np.concatenate(outs, axis=1).astype(np.float32)


if __name__ == "__main__":
    rng = np.random.default_rng(0)
    spt = np.cumsum(rng.uniform(0.1, 5.0, (S, B_FULL)).astype(np.float32), axis=0)
    wt = np.asarray(
        [0.176786766570677, 0.216967308403809, 0.254893976981164,
         -0.704205679427144, 0.025], dtype=np.float32)
    o = kernel(spt, wt)
    print(o.shape, o.dtype, o[:3, :3])


# revision 3
# speedup vs baseline: 2.2156x; 2.2156x over previous
"""ACT-R activation recurrence kernel for 8 TRN2 NeuronCores — transposed layout.

Math (per batch column b, S=128 steps):
  m[0] = -inf;  decay[j] = c*exp(m[j]) + a
  m[i] = log(sum_{j<i} ((sp[i]-sp[j])*scale)^(-decay[j])),  scale = 86400*h
  out[i-1] = sigmoid((m[i] - tau)/s)

Layout: batch on partitions. Per core B=2048 = 128 partitions x 16 chunks; the
step axis j lives on the free axis: X[p, bc, j] ~ x[j, bc*128+p].  All engine
costs on TRN2 scale with FREE size only, so this halves elementwise cost vs
the j-on-partition layout and eliminates every matmul:
  d_i[:, :, j<i] = sp_i - sp_j        (broadcast-sub, Pool engine, f32)
  L_i = Ln(scale*d_i)                 (ACT, f16 out) — computed W steps ahead
  Q_i = L_i * negdecay[:, :, j<i]     (DVE f16, 2x mode)
  T_i = Exp(Q_i)                      (ACT f16)
  P_i = reduce_add_X(T_i[j<i-1])      (DVE, f32)  [bulk, lag-1]
  F_i = Exp(L_i[i-1]*negdecay[i-1])   (tiny [128,16,1] critical-path ops)
  negdecay[:, :, i] = -c*(P_i + F_i) - a   (fused scalar_tensor_tensor)
Final: S = (negdecay + a)/(-c); out = Sigmoid((Ln(S) - tau)/s).

Ln and Exp share one activation table (natural_log_exp_and_others — steered via
a get_activation_tables patch), so there is no per-step table reload.
No PSUM, no PE, no collectives.  Outputs are concatenated on host.
"""

import sys

for _p in ("/opt/trn_rl_repo",):
    if _p not in sys.path:
        sys.path.insert(0, _p)

import numpy as np
from contextlib import ExitStack

# --- act table patch: serve Ln and Exp from the combined table so the ---
# --- per-step Ln/Exp alternation does not thrash ACT table loads.     ---
import concourse.hw_specs as hw_specs
import concourse.bacc as bacc_mod
from concourse import mybir

_orig_gat = hw_specs.get_activation_tables


def _patched_gat(arch):
    tabs = _orig_gat(arch)
    out = {}
    ln_t = mybir.ActivationFunctionType.Ln
    exp_t = mybir.ActivationFunctionType.Exp
    for name, funcs in tabs.items():
        f = set(funcs)
        if name != "natural_log_exp_and_others":
            f.discard(ln_t)
            f.discard(exp_t)
        out[name] = f
    return out


bacc_mod.get_activation_tables = _patched_gat

import concourse.bass as bass
import concourse.bacc as bacc
import concourse.tile as tile
from concourse.bass_utils import run_bass_kernel_spmd

S = 128
B_FULL = 16384
N_CORES = 8
B = B_FULL // N_CORES  # 2048 per core
P = 128                # partitions (batch)
NC = B // P            # 16 chunks per partition

F32 = mybir.dt.float32
F16 = mybir.dt.float16
AF = mybir.ActivationFunctionType
ALU = mybir.AluOpType

W = 4      # Ln lookahead (steps)
NR = W + 1  # ring slots for D / L


def build_kernel(a, c, s, tau, h, repeat=1):
    scale = 86400.0 * float(h)
    nc = bacc.Bacc()

    spt_in = nc.declare_dram_parameter("spt", [P, NC, S], F32, isOutput=False)
    out_ext = nc.declare_dram_parameter("out", [P, NC, S], F32, isOutput=True)

    with ExitStack() as ctx:
        tc = ctx.enter_context(tile.TileContext(nc))
        pool = ctx.enter_context(tc.tile_pool(name="p", bufs=1))

        SPT = pool.tile([P, NC, S], F32)
        nc.sync.dma_start(out=SPT[:], in_=spt_in[:])

        # negdecay[:, :, j] = -(a + c*S_j); col 0 = -a (S_0 = exp(-inf) = 0)
        NEG = pool.tile([P, NC, S], F16)
        # NEGP[:, :, i] = -c*P_i - a (bulk partial, written for i >= 2;
        # memset covers i = 1 where P_1 = 0)
        NEGP = pool.tile([P, NC, S], F32)

        D = [pool.tile([P, NC, S], F32, name=f"D{r}") for r in range(NR)]
        LR = [pool.tile([P, NC, S], F16, name=f"LR{r}") for r in range(NR)]
        QB = [pool.tile([P, NC, S], F16, name=f"QB{r}") for r in range(2)]
        TB = [pool.tile([P, NC, S], F16, name=f"TB{r}") for r in range(2)]
        QF = [pool.tile([P, NC, 1], F16, name=f"QF{r}") for r in range(2)]
        TF = [pool.tile([P, NC, 1], F32, name=f"TF{r}") for r in range(2)]
        PR = [pool.tile([P, NC, 1], F32, name=f"PR{r}") for r in range(2)]
        BIAS = pool.tile([P, 1], F32)
        nc.vector.memset(BIAS[:], -float(tau) / float(s))

        def emit_lookahead(i):
            # D_i[:, :, 0:i] = sp_i - sp_j ; L_i = Ln(scale * D_i)   (f16)
            r = i % NR
            b0, b1 = bass.broadcast_tensor_aps(
                SPT[:, :, i : i + 1], SPT[:, :, 0:i]
            )
            nc.gpsimd.tensor_tensor(
                out=D[r][:, :, 0:i], in0=b0, in1=b1, op=ALU.subtract
            )
            nc.scalar.activation(
                LR[r][:, :, 0:i], D[r][:, :, 0:i], AF.Ln, scale=scale
            )

        for _rep in range(repeat):
            nc.vector.memset(NEG[:], -float(a))
            nc.vector.memset(NEGP[:], -float(a))

            for i in range(1, min(W + 1, S)):
                emit_lookahead(i)

            for i in range(1, S):
                r = i % NR
                x = i % 2
                # --- fresh term (critical path): F_i = exp(L[i-1]*neg[i-1]) ---
                nc.vector.tensor_tensor(
                    out=QF[x][:], in0=LR[r][:, :, i - 1 : i],
                    in1=NEG[:, :, i - 1 : i], op=ALU.mult,
                )
                nc.scalar.activation(TF[x][:], QF[x][:], AF.Exp)
                # --- bulk for step i+1: P_{i+1} = sum_{j<i} T ---
                if i + 1 < S:
                    r1 = (i + 1) % NR
                    x1 = (i + 1) % 2
                    nc.vector.tensor_tensor(
                        out=QB[x1][:, :, 0:i], in0=LR[r1][:, :, 0:i],
                        in1=NEG[:, :, 0:i], op=ALU.mult,
                    )
                    nc.scalar.activation(
                        TB[x1][:, :, 0:i], QB[x1][:, :, 0:i], AF.Exp
                    )
                # --- close step i: NEG[i] = -c*TF + NEGP[i] ---
                nc.vector.scalar_tensor_tensor(
                    out=NEG[:, :, i : i + 1], in0=TF[x][:], scalar=-float(c),
                    in1=NEGP[:, :, i : i + 1], op0=ALU.mult, op1=ALU.add,
                )
                if i + 1 < S:
                    nc.vector.tensor_reduce(
                        out=PR[x1][:], in_=TB[x1][:, :, 0:i],
                        axis=mybir.AxisListType.X, op=ALU.add,
                    )
                    nc.vector.tensor_scalar(
                        out=NEGP[:, :, i + 1 : i + 2], in0=PR[x1][:],
                        scalar1=-float(c), scalar2=-float(a),
                        op0=ALU.mult, op1=ALU.add,
                    )
                if i + W < S:
                    emit_lookahead(i + W)

        # --- epilogue: S = (NEG + a)*(-1/c); out = Sigmoid((Ln(S)-tau)/s) ---
        SS = pool.tile([P, NC, S], F32)
        nc.vector.tensor_scalar(
            out=SS[:, :, 1:S], in0=NEG[:, :, 1:S],
            scalar1=float(a), scalar2=-1.0 / float(c),
            op0=ALU.add, op1=ALU.mult,
        )
        M = pool.tile([P, NC, S], F32)
        nc.scalar.activation(M[:, :, 1:S], SS[:, :, 1:S], AF.Ln)
        O = pool.tile([P, NC, S], F32)
        nc.vector.memset(O[:, :, 0:1], 0.0)
        nc.scalar.activation(
            O[:, :, 1:S], M[:, :, 1:S], AF.Sigmoid,
            scale=1.0 / float(s), bias=BIAS[:],
        )
        nc.sync.dma_start(out=out_ext[:], in_=O[:])

    nc.compile()
    return nc


def make_in_maps(sp: np.ndarray) -> list:
    in_maps = []
    for ci in range(N_CORES):
        shard = sp[:, ci * B : (ci + 1) * B]  # [S, B]
        # spt[p, bc, j] = shard[j, bc*128 + p]
        spt = np.ascontiguousarray(
            shard.reshape(S, NC, P).transpose(2, 1, 0).astype(np.float32)
        )
        in_maps.append({"spt": spt})
    return in_maps


def kernel(sp: np.ndarray, w: np.ndarray) -> np.ndarray:
    sp = np.ascontiguousarray(sp, dtype=np.float32)
    w = np.asarray(w, dtype=np.float32)
    a, c, s, tau, h = (float(x) for x in w)

    nc = build_kernel(a, c, s, tau, h)
    in_maps = make_in_maps(sp)

    res = run_bass_kernel_spmd(nc, in_maps, core_ids=list(range(N_CORES)))
    outs = []
    for ci in range(N_CORES):
        o = res.results[ci]["out"]  # [P, NC, S]
        outs.append(o.transpose(2, 1, 0).reshape(S, B)[1:S])
    return np.concatenate(outs, axis=1).astype(np.float32)


if __name__ == "__main__":
    rng = np.random.default_rng(0)
    spt = np.cumsum(rng.uniform(0.1, 5.0, (S, B_FULL)).astype(np.float32), axis=0)
    wt = np.asarray(
        [0.176786766570677, 0.216967308403809, 0.254893976981164,
         -0.704205679427144, 0.025], dtype=np.float32)
    o = kernel(spt, wt)
    print(o.shape, o.dtype, o[:3, :3])
